# revision 5
# baseline (speedup 1.0000x reference)
"""Kuramoto oscillator network kernel for 8 Trainium2 NeuronCores.

Problem: B=256 batches, D=256 feature dims, N=16 oscillator dims, T=25 steps.
    c = emb[:,:,None]*W_d + b_d                        [B,D,N]
    x = normalize(noise + c)                            (init, per (b,d) over N)
    repeat T: f1 = J_in@x1 + J_out@x2 + c1  (einsum ijkl,bjl->bik)
              p  = f - <x,f>x ; om = Omega@x
              x  = normalize(x + g*(om + p))
    out = stack(x1, x2)                                 [2,B,D,N]

Strategy (model-parallel over output dim i, all-transposed layout):
  * Each core owns a 512-wide slice of the flattened ik axis (32 of 256 i's).
    J_in^T / J_out^T slices ([4096 x 512] each) stay resident in SBUF and are
    used as matmul stationary tiles in float32r (FP22 read truncation, full
    bf16-rate on the PE vs 4x slower true fp32).
  * State X^T [(j,l), batchcol] lives in HBM, AllGather'd across cores each
    step; columns are [x1|x2] so the J_out cross-coupling is a column-swapped
    rhs view (negative-stride AP).
  * Omega rotation is 4 extra block-diagonal matmuls accumulated into the same
    PSUM as f (skew-symmetry makes <x,Omega x>=0, so the tangent projection is
    unaffected).
  * Per-(b,i) reductions over the 16 oscillator partitions (projection <x,f>
    and the normalize norm) are single matmuls against a block-ones matrix
    that reduce AND broadcast in one shot.
  * Batches split into two groups (A=0:128, B=128:256) pipelined so each
    group's AllGather hides under the other group's matmul phase.

Self-contained: hardcodes shapes; no imports from /root/problem.
"""

import os
import sys
import time

sys.path.insert(0, "/opt/trn_rl_repo")

import numpy as np

import concourse.bass as bass
import concourse.mybir as mybir
import concourse.tile as tile
from concourse import bacc
from concourse import bass2jax
from concourse.bass_interp import get_hw_module

B, D, N = 256, 256, 16
DN = D * N                      # 4096 flattened (i,k) / (j,l)
T = int(os.environ.get("KUR_T", "25"))
GAMMA = 0.1
NCORES = 8
IKS = DN // NCORES              # 512 ik per core (32 i values)
NIPC = D // NCORES              # 32 i per core
GCOL = 256                      # columns per batch group (128 x1 + 128 x2)
HALF = 128

FP32 = mybir.dt.float32
FP32R = mybir.dt.float32r

_CACHE = {}


def _swap_halves(ap):
    """View a [128, 256] SBUF AP with its two 128-column halves swapped."""
    return bass.AP(
        tensor=ap.tensor,
        offset=ap.offset + HALF,
        ap=[list(ap.ap[0])] + [[-HALF, 2], [1, HALF]],
    )


def _build(nc):
    AF = mybir.ActivationFunctionType

    # ---------------- DRAM I/O ----------------
    jt_in_d = nc.dram_tensor("jt_in", [DN, IKS], FP32R, kind="ExternalInput")
    jt_out_d = nc.dram_tensor("jt_out", [DN, IKS], FP32R, kind="ExternalInput")
    ombd_d = nc.dram_tensor("ombd", [4 * HALF, HALF], FP32R, kind="ExternalInput")
    wdiag_d = nc.dram_tensor("wdiag", [32 * HALF, HALF], FP32, kind="ExternalInput")
    wdiag_s_d = nc.dram_tensor("wdiag_s", [4 * HALF, HALF], FP32, kind="ExternalInput")
    bones_d = nc.dram_tensor("bones", [HALF, HALF], FP32, kind="ExternalInput")
    embt_d = nc.dram_tensor("embt", [2 * HALF, 2 * GCOL], FP32, kind="ExternalInput")
    embt_own_d = nc.dram_tensor("embt_own", [HALF, 2 * GCOL], FP32, kind="ExternalInput")
    bdt_d = nc.dram_tensor("bdt", [HALF, 32], FP32, kind="ExternalInput")
    bdt_s_d = nc.dram_tensor("bdt_s", [HALF, 4], FP32, kind="ExternalInput")
    noiset_d = nc.dram_tensor("noiset", [DN, 2 * GCOL], FP32, kind="ExternalInput")
    noiset_own_d = nc.dram_tensor("noiset_own", [IKS, 2 * GCOL], FP32, kind="ExternalInput")

    xt_out_d = nc.dram_tensor("xt_out", [DN, 2 * GCOL], FP32, kind="ExternalOutput")
    DBG = os.environ.get("KUR_DBG", "0") == "1"
    if DBG:
        dbg_h_d = nc.dram_tensor("dbg_h", [HALF, GCOL], FP32, kind="ExternalOutput")

    # internal HBM: gathered state + AG input bounce, per group
    xg_t = [
        nc.dram_tensor(f"xg{g}_t", [DN, GCOL], FP32, addr_space="Shared")
        for g in range(2)
    ]
    agin = [nc.dram_tensor(f"agin{g}", [IKS, GCOL], FP32) for g in range(2)]

    with tile.TileContext(nc) as tc:
        with (
            tc.tile_pool(name="res", bufs=1) as res,
            tc.tile_pool(name="stream", bufs=4) as stream,
            tc.tile_pool(name="xstream", bufs=5) as xstream,
            tc.tile_pool(name="tmp", bufs=2) as tmp,
            tc.tile_pool(name="fps", bufs=2, space="PSUM") as fps,
            tc.tile_pool(name="auxps", bufs=3, space="PSUM") as auxps,
            tc.tile_pool(name="dram", bufs=1, space="DRAM") as _dr,
        ):
            # ---------------- resident SBUF ----------------
            j_in_sb = res.tile([HALF, 32 * IKS], FP32R, tag="jin")
            j_out_sb = res.tile([HALF, 32 * IKS], FP32R, tag="jout")
            for k in range(32):
                nc.sync.dma_start(
                    out=j_in_sb[:, k * IKS:(k + 1) * IKS],
                    in_=jt_in_d[k * HALF:(k + 1) * HALF, :],
                )
                nc.sync.dma_start(
                    out=j_out_sb[:, k * IKS:(k + 1) * IKS],
                    in_=jt_out_d[k * HALF:(k + 1) * HALF, :],
                )

            ombd_sb = res.tile([HALF, 4 * HALF], FP32R, tag="ombd")
            for m in range(4):
                nc.sync.dma_start(
                    out=ombd_sb[:, m * HALF:(m + 1) * HALF],
                    in_=ombd_d[m * HALF:(m + 1) * HALF, :],
                )
            bones_r = res.tile([HALF, HALF], FP32R, tag="bones_r")
            bones_f = res.tile([HALF, HALF], FP32, tag="bones_f")
            nc.sync.dma_start(out=bones_r[:, :], in_=bones_d[:, :].bitcast(FP32R))
            nc.sync.dma_start(out=bones_f[:, :], in_=bones_d[:, :])

            embt_sb = res.tile([HALF, 2 * 2 * GCOL], FP32, tag="embt")
            for r in range(2):
                nc.sync.dma_start(
                    out=embt_sb[:, r * 2 * GCOL:(r + 1) * 2 * GCOL],
                    in_=embt_d[r * HALF:(r + 1) * HALF, :],
                )
            embt_own_sb = res.tile([HALF, 2 * GCOL], FP32, tag="embt_own")
            nc.sync.dma_start(out=embt_own_sb[:, :], in_=embt_own_d[:, :])
            bdt_sb = res.tile([HALF, 32], FP32, tag="bdt")
            nc.sync.dma_start(out=bdt_sb[:, :], in_=bdt_d[:, :])
            bdt_s_sb = res.tile([HALF, 4], FP32, tag="bdt_s")
            nc.sync.dma_start(out=bdt_s_sb[:, :], in_=bdt_s_d[:, :])

            # state slices (own ik rows), fp32 + rounded fp32r copy
            xs = res.tile([HALF, 8 * GCOL], FP32, tag="xs")
            xsr = res.tile([HALF, 8 * GCOL], FP32R, tag="xsr")
            cs = res.tile([HALF, 8 * GCOL], FP32, tag="cs")

            # ---------------- init: full X0 -> xg_t ----------------
            def normalize_into(u2, bones_tile, dt_rhs, out_sl_writer):
                """u2: [128,256] fp32 SBUF tile. Writes normalized result."""
                sq = tmp.tile([HALF, GCOL], dt_rhs, tag="sq")
                nc.vector.tensor_mul(out=sq[:, :], in0=u2[:, :], in1=u2[:, :])
                nb = auxps.tile([HALF, GCOL], FP32, tag="aux")
                nc.tensor.matmul(
                    nb[:, :], bones_tile[:, :], sq[:, :], start=True, stop=True
                )
                nrm = tmp.tile([HALF, GCOL], FP32, tag="nrm")
                nc.scalar.activation(nrm[:, :], nb[:, :], AF.Sqrt)
                rinv = tmp.tile([HALF, GCOL], FP32, tag="rinv")
                nc.vector.reciprocal(out=rinv[:, :], in_=nrm[:, :])
                out_sl_writer(u2, rinv)

            for g in range(2):
                for kk in range(32):
                    cps = auxps.tile([HALF, GCOL], FP32, tag="aux")
                    wt = stream.tile([HALF, HALF], FP32, tag="wdiag")
                    nc.sync.dma_start(
                        out=wt[:, :], in_=wdiag_d[kk * HALF:(kk + 1) * HALF, :]
                    )
                    nc.tensor.matmul(
                        cps[:, :],
                        wt[:, :],
                        embt_sb[:, (kk // 16) * 2 * GCOL + g * GCOL:][:, :GCOL],
                        start=True,
                        stop=True,
                    )
                    nt = stream.tile([HALF, GCOL], FP32, tag="noise")
                    nc.sync.dma_start(
                        out=nt[:, :],
                        in_=noiset_d[kk * HALF:(kk + 1) * HALF, g * GCOL:(g + 1) * GCOL],
                    )
                    u2 = tmp.tile([HALF, GCOL], FP32, tag="u2")
                    # u2 = (cps + b_d_col) + noise
                    nc.vector.scalar_tensor_tensor(
                        out=u2[:, :],
                        in0=cps[:, :],
                        scalar=bdt_sb[:, kk:kk + 1],
                        in1=nt[:, :],
                        op0=mybir.AluOpType.add,
                        op1=mybir.AluOpType.add,
                    )

                    def wr(u2_, rinv_, g=g, kk=kk):
                        x0 = tmp.tile([HALF, GCOL], FP32, tag="x0")
                        nc.vector.tensor_mul(out=x0[:, :], in0=u2_[:, :], in1=rinv_[:, :])
                        nc.sync.dma_start(
                            out=xg_t[g][kk * HALF:(kk + 1) * HALF, :], in_=x0[:, :]
                        )

                    normalize_into(u2, bones_f, FP32, wr)

            # ---------------- init: own-slice c_s and x_s ----------------
            for g in range(2):
                for m in range(4):
                    sl = slice((g * 4 + m) * GCOL, (g * 4 + m + 1) * GCOL)
                    cps = auxps.tile([HALF, GCOL], FP32, tag="aux")
                    wt = stream.tile([HALF, HALF], FP32, tag="wdiag")
                    nc.sync.dma_start(
                        out=wt[:, :], in_=wdiag_s_d[m * HALF:(m + 1) * HALF, :]
                    )
                    nc.tensor.matmul(
                        cps[:, :],
                        wt[:, :],
                        embt_own_sb[:, g * GCOL:(g + 1) * GCOL],
                        start=True,
                        stop=True,
                    )
                    nc.vector.tensor_scalar_add(
                        cs[:, sl], cps[:, :], bdt_s_sb[:, m:m + 1]
                    )
                    nt = stream.tile([HALF, GCOL], FP32, tag="noise")
                    nc.sync.dma_start(
                        out=nt[:, :],
                        in_=noiset_own_d[m * HALF:(m + 1) * HALF, g * GCOL:(g + 1) * GCOL],
                    )
                    u2 = tmp.tile([HALF, GCOL], FP32, tag="u2")
                    nc.vector.tensor_add(out=u2[:, :], in0=cs[:, sl], in1=nt[:, :])

                    def wr(u2_, rinv_, sl=sl):
                        nc.vector.tensor_mul(out=xs[:, sl], in0=u2_[:, :], in1=rinv_[:, :])
                        nc.vector.tensor_copy(out=xsr[:, sl], in_=xs[:, sl])

                    normalize_into(u2, bones_f, FP32, wr)

            # ---------------- main loop ----------------
            prev_cc = [None, None]
            for t in range(T):
                for g in range(2):
                    f01 = fps.tile([HALF, 2 * GCOL], FP32, tag=f"f{g}")
                    f23 = fps.tile([HALF, 2 * GCOL], FP32, tag=f"f{g}")

                    def freg(m):
                        ft = f01 if m < 2 else f23
                        c0 = (m % 2) * GCOL
                        return ft[:, c0:c0 + GCOL]

                    for k in range(32):
                        xk = xstream.tile([HALF, GCOL], FP32R, tag=f"xg{g}")
                        dma = nc.sync.dma_start(
                            out=xk[:, :],
                            in_=xg_t[g][k * HALF:(k + 1) * HALF, :].bitcast(FP32R),
                        )
                        if prev_cc[g] is not None:
                            tile.add_dep_helper(
                                dma.ins, prev_cc[g].ins, reason="AG->stream RAW"
                            )
                        xk_sw = _swap_halves(xk[:, :])
                        for m in range(4):
                            # start=True clears the whole PSUM bank's
                            # has_written bits -- issue it only on the first
                            # matmul ever touching each bank (m=0/m=2, k=0).
                            nc.tensor.matmul(
                                freg(m),
                                j_in_sb[:, k * IKS + m * HALF:][:, :HALF],
                                xk[:, :],
                                start=(k == 0 and m % 2 == 0),
                                stop=False,
                                skip_group_check=True,
                            )
                            nc.tensor.matmul(
                                freg(m),
                                j_out_sb[:, k * IKS + m * HALF:][:, :HALF],
                                xk_sw,
                                start=False,
                                stop=False,
                                skip_group_check=True,
                            )

                    for m in range(4):
                        sl = slice((g * 4 + m) * GCOL, (g * 4 + m + 1) * GCOL)
                        # Omega rotation into same accumulation
                        nc.tensor.matmul(
                            freg(m),
                            ombd_sb[:, m * HALF:(m + 1) * HALF],
                            xsr[:, sl],
                            start=False,
                            stop=(m % 2 == 1),
                            skip_group_check=True,
                        )
                        h = tmp.tile([HALF, GCOL], FP32, tag="h")
                        nc.vector.tensor_add(out=h[:, :], in0=freg(m), in1=cs[:, sl])
                        if DBG and t == 0 and g == 0 and m == 0:
                            nc.sync.dma_start(out=dbg_h_d[:, :], in_=h[:, :])
                        xf = tmp.tile([HALF, GCOL], FP32R, tag="xf")
                        nc.vector.tensor_mul(out=xf[:, :], in0=h[:, :], in1=xs[:, sl])
                        sb_ps = auxps.tile([HALF, GCOL], FP32, tag="aux")
                        nc.tensor.matmul(
                            sb_ps[:, :], bones_r[:, :], xf[:, :], start=True, stop=True
                        )
                        t1 = tmp.tile([HALF, GCOL], FP32, tag="t1")
                        nc.vector.scalar_tensor_tensor(
                            out=t1[:, :],
                            in0=sb_ps[:, :],
                            scalar=GAMMA,
                            in1=xs[:, sl],
                            op0=mybir.AluOpType.mult,
                            op1=mybir.AluOpType.mult,
                        )
                        t2 = tmp.tile([HALF, GCOL], FP32, tag="t2")
                        nc.vector.scalar_tensor_tensor(
                            out=t2[:, :],
                            in0=h[:, :],
                            scalar=GAMMA,
                            in1=xs[:, sl],
                            op0=mybir.AluOpType.mult,
                            op1=mybir.AluOpType.add,
                        )
                        pre = tmp.tile([HALF, GCOL], FP32, tag="pre")
                        nc.vector.tensor_sub(out=pre[:, :], in0=t2[:, :], in1=t1[:, :])

                        def wr(pre_, rinv_, sl=sl, m=m, g=g):
                            nc.vector.tensor_mul(
                                out=xs[:, sl], in0=pre_[:, :], in1=rinv_[:, :]
                            )
                            nc.vector.tensor_copy(out=xsr[:, sl], in_=xs[:, sl])
                            nc.sync.dma_start(
                                out=agin[g][m * HALF:(m + 1) * HALF, :], in_=xs[:, sl]
                            )

                        normalize_into(pre, bones_r, FP32R, wr)

                    cc = nc.gpsimd.collective_compute(
                        "AllGather",
                        mybir.AluOpType.bypass,
                        replica_groups=[list(range(NCORES))],
                        ins=[agin[g][:, :].opt()],
                        outs=[xg_t[g][:, :].opt()],
                    )
                    prev_cc[g] = cc

            # ---------------- output ----------------
            for g in range(2):
                dma = nc.sync.dma_start(
                    out=xt_out_d[:, g * GCOL:(g + 1) * GCOL], in_=xg_t[g][:, :]
                )
                if prev_cc[g] is not None:
                    tile.add_dep_helper(dma.ins, prev_cc[g].ins, reason="AG->out RAW")

    nc.compile()
    return nc


def _get_nc():
    if "nc" not in _CACHE:
        nc = bacc.Bacc(
            "TRN2", target_bir_lowering=False, debug=False, num_devices=NCORES
        )
        _build(nc)
        nc.m = get_hw_module(nc.m)
        _CACHE["nc"] = nc
    return _CACHE["nc"]


def _marshal(embeddings1, embeddings2, W_d, b_d, J_in, J_out, Omega, noise1, noise2):
    """Host-side pure data movement: slice/transpose/scatter into device layout."""
    f32 = np.float32

    # J^T[(j,l),(i,k)] = J[i,j,k,l]
    jt_in = np.ascontiguousarray(J_in.transpose(1, 3, 0, 2).reshape(DN, DN))
    jt_out = np.ascontiguousarray(J_out.transpose(1, 3, 0, 2).reshape(DN, DN))

    # column layout: [x1_A | x2_A | x1_B | x2_B], 128 batches each
    def colcat(a1, a2):  # a1,a2: [B, ...] -> stacked columns
        return np.concatenate(
            [a1[:HALF], a2[:HALF], a1[HALF:], a2[HALF:]], axis=0
        )

    embt = np.ascontiguousarray(colcat(embeddings1, embeddings2).T)      # [256,512]
    nt1 = noise1.transpose(1, 2, 0).reshape(DN, B)                       # [(d,n), b]
    nt2 = noise2.transpose(1, 2, 0).reshape(DN, B)
    noiset = np.ascontiguousarray(
        np.concatenate([nt1[:, :HALF], nt2[:, :HALF], nt1[:, HALF:], nt2[:, HALF:]], axis=1)
    )

    bdt_flat = np.ascontiguousarray(b_d.reshape(DN))
    bdt = np.ascontiguousarray(bdt_flat.reshape(32, HALF).T)             # [128, 32]

    # W_diag scatter: block kk covers i in [8kk, 8kk+8)
    wdiag = np.zeros((32, HALF, HALF), f32)
    for kk in range(32):
        base_row = 128 * (kk // 16)
        for iloc in range(8):
            i = 8 * kk + iloc
            wdiag[kk, i - base_row, iloc * 16:(iloc + 1) * 16] = W_d[i]
    wdiag = wdiag.reshape(32 * HALF, HALF)

    bones = np.kron(np.eye(8, dtype=f32), np.ones((16, 16), f32))

    in_maps = []
    for c in range(NCORES):
        i0 = NIPC * c
        ik0 = IKS * c
        ombd = np.zeros((4, HALF, HALF), f32)
        for m in range(4):
            for iloc in range(8):
                i = i0 + 8 * m + iloc
                ombd[m, iloc * 16:(iloc + 1) * 16, iloc * 16:(iloc + 1) * 16] = Omega[i].T
        in_maps.append(
            {
                "jt_in": np.ascontiguousarray(jt_in[:, ik0:ik0 + IKS]),
                "jt_out": np.ascontiguousarray(jt_out[:, ik0:ik0 + IKS]),
                "ombd": ombd.reshape(4 * HALF, HALF),
                "wdiag": wdiag,
                "wdiag_s": np.ascontiguousarray(
                    wdiag.reshape(32, HALF, HALF)[4 * c:4 * c + 4].reshape(4 * HALF, HALF)
                ),
                "bones": bones,
                "embt": embt,
                "embt_own": np.ascontiguousarray(
                    embt[128 * (c // 4):128 * (c // 4) + 128]
                ),
                "bdt": bdt,
                "bdt_s": np.ascontiguousarray(
                    bdt_flat[ik0:ik0 + IKS].reshape(4, HALF).T
                ),
                "noiset": noiset,
                "noiset_own": np.ascontiguousarray(noiset[ik0:ik0 + IKS]),
            }
        )
    return in_maps


def _unmarshal(xt):
    """xt: [4096, 512] -> [2, B, D, N]"""
    x1 = np.concatenate([xt[:, 0:HALF], xt[:, 2 * HALF:3 * HALF]], axis=1)
    x2 = np.concatenate([xt[:, HALF:2 * HALF], xt[:, 3 * HALF:4 * HALF]], axis=1)
    out = np.empty((2, B, D, N), np.float32)
    out[0] = x1.reshape(D, N, B).transpose(2, 0, 1)
    out[1] = x2.reshape(D, N, B).transpose(2, 0, 1)
    return out


def run_on_device(in_maps):
    nc = _get_nc()
    return bass2jax.run_bass_via_pjrt(nc, in_maps, n_cores=NCORES)


def kernel(**inputs):
    in_maps = _marshal(**{k: np.asarray(v, np.float32) for k, v in inputs.items()})
    results = run_on_device(in_maps)
    return _unmarshal(results[0]["xt_out"])


if __name__ == "__main__":
    rng = np.random.default_rng(0)
    ins = {
        "embeddings1": rng.standard_normal((B, D), dtype=np.float32),
        "embeddings2": rng.standard_normal((B, D), dtype=np.float32),
        "W_d": rng.standard_normal((D, N), dtype=np.float32) * 0.1,
        "b_d": np.zeros((D, N), np.float32),
        "J_in": (rng.standard_normal((D, D, N, N), dtype=np.float32) * 0.007),
        "J_out": (rng.standard_normal((D, D, N, N), dtype=np.float32) * 0.007),
        "Omega": rng.standard_normal((D, N, N), dtype=np.float32) * 0.1,
        "noise1": rng.standard_normal((B, D, N), dtype=np.float32) * 0.05,
        "noise2": rng.standard_normal((B, D, N), dtype=np.float32) * 0.05,
    }
    t0 = time.time()
    out = kernel(**ins)
    print("kernel() took", time.time() - t0, "s; out shape", out.shape)


# revision 7
# speedup vs baseline: 1.1762x; 1.1762x over previous
"""Kuramoto oscillator network kernel for 8 Trainium2 NeuronCores.

Problem: B=256 batches, D=256 feature dims, N=16 oscillator dims, T=25 steps.
    c = emb[:,:,None]*W_d + b_d                        [B,D,N]
    x = normalize(noise + c)                            (init, per (b,d) over N)
    repeat T: f1 = J_in@x1 + J_out@x2 + c1  (einsum ijkl,bjl->bik)
              p  = f - <x,f>x ; om = Omega@x
              x  = normalize(x + g*(om + p))
    out = stack(x1, x2)                                 [2,B,D,N]

Strategy (model-parallel over output dim i, all-transposed layout):
  * Each core owns a 512-wide slice of the flattened ik axis (32 of 256 i's).
    J_in^T / J_out^T slices ([4096 x 512] each) stay resident in SBUF and are
    used as matmul stationary tiles in float32r (FP22 read truncation, full
    bf16-rate on the PE vs 4x slower true fp32).
  * State X^T [(j,l), batchcol] lives in HBM, AllGather'd across cores each
    step; columns are [x1|x2] so the J_out cross-coupling is a column-swapped
    rhs view (negative-stride AP).
  * Omega rotation is 4 extra block-diagonal matmuls accumulated into the same
    PSUM as f (skew-symmetry makes <x,Omega x>=0, so the tangent projection is
    unaffected).
  * Per-(b,i) reductions over the 16 oscillator partitions (projection <x,f>
    and the normalize norm) are single matmuls against a block-ones matrix
    that reduce AND broadcast in one shot.
  * Batches split into two groups (A=0:128, B=128:256) pipelined so each
    group's AllGather hides under the other group's matmul phase.

Self-contained: hardcodes shapes; no imports from /root/problem.
"""

import os
import sys
import time

sys.path.insert(0, "/opt/trn_rl_repo")

import numpy as np

import concourse.bass as bass
import concourse.mybir as mybir
import concourse.tile as tile
from concourse import bacc
from concourse import bass2jax
from concourse.bass_interp import get_hw_module

B, D, N = 256, 256, 16
DN = D * N                      # 4096 flattened (i,k) / (j,l)
T = int(os.environ.get("KUR_T", "25"))
GAMMA = 0.1
NCORES = 8
IKS = DN // NCORES              # 512 ik per core (32 i values)
NIPC = D // NCORES              # 32 i per core
GCOL = 256                      # columns per batch group (128 x1 + 128 x2)
HALF = 128

FP32 = mybir.dt.float32
FP32R = mybir.dt.float32r
FP16 = mybir.dt.float16

_CACHE = {}


def _swap_halves(ap):
    """View a [128, 256] SBUF AP with its two 128-column halves swapped."""
    return bass.AP(
        tensor=ap.tensor,
        offset=ap.offset + HALF,
        ap=[list(ap.ap[0])] + [[-HALF, 2], [1, HALF]],
    )


def _build(nc):
    AF = mybir.ActivationFunctionType

    # ---------------- DRAM I/O ----------------
    jt_in_d = nc.dram_tensor("jt_in", [DN, IKS], FP16, kind="ExternalInput")
    jt_out_d = nc.dram_tensor("jt_out", [DN, IKS], FP16, kind="ExternalInput")
    ombd_d = nc.dram_tensor("ombd", [4 * HALF, HALF], FP32R, kind="ExternalInput")
    wdiag_d = nc.dram_tensor("wdiag", [32 * HALF, HALF], FP32, kind="ExternalInput")
    wdiag_s_d = nc.dram_tensor("wdiag_s", [4 * HALF, HALF], FP32, kind="ExternalInput")
    bones_d = nc.dram_tensor("bones", [HALF, HALF], FP32, kind="ExternalInput")
    embt_d = nc.dram_tensor("embt", [2 * HALF, 2 * GCOL], FP32, kind="ExternalInput")
    embt_own_d = nc.dram_tensor("embt_own", [HALF, 2 * GCOL], FP32, kind="ExternalInput")
    bdt_d = nc.dram_tensor("bdt", [HALF, 32], FP32, kind="ExternalInput")
    bdt_s_d = nc.dram_tensor("bdt_s", [HALF, 4], FP32, kind="ExternalInput")
    noiset_d = nc.dram_tensor("noiset", [DN, 2 * GCOL], FP32, kind="ExternalInput")
    noiset_own_d = nc.dram_tensor("noiset_own", [IKS, 2 * GCOL], FP32, kind="ExternalInput")

    xt_out_d = nc.dram_tensor("xt_out", [IKS, 2 * GCOL], FP32, kind="ExternalOutput")
    DBG = os.environ.get("KUR_DBG", "0") == "1"
    if DBG:
        dbg_h_d = nc.dram_tensor("dbg_h", [HALF, GCOL], FP32, kind="ExternalOutput")

    # internal HBM: gathered state + AG input bounce, per group
    xg_t = [
        nc.dram_tensor(f"xg{g}_t", [DN, GCOL], FP16, addr_space="Shared")
        for g in range(2)
    ]
    agin = [nc.dram_tensor(f"agin{g}", [IKS, GCOL], FP16) for g in range(2)]

    with tile.TileContext(nc) as tc:
        with (
            tc.tile_pool(name="res", bufs=1) as res,
            tc.tile_pool(name="stream", bufs=4) as stream,
            tc.tile_pool(name="xstream", bufs=5) as xstream,
            tc.tile_pool(name="tmp", bufs=2) as tmp,
            tc.tile_pool(name="fps", bufs=2, space="PSUM") as fps,
            tc.tile_pool(name="auxps", bufs=3, space="PSUM") as auxps,
            tc.tile_pool(name="dram", bufs=1, space="DRAM") as _dr,
        ):
            # ---------------- resident SBUF ----------------
            j_in_sb = res.tile([HALF, 32 * IKS], FP16, tag="jin")
            j_out_sb = res.tile([HALF, 32 * IKS], FP16, tag="jout")
            for k in range(32):
                nc.sync.dma_start(
                    out=j_in_sb[:, k * IKS:(k + 1) * IKS],
                    in_=jt_in_d[k * HALF:(k + 1) * HALF, :],
                )
                nc.sync.dma_start(
                    out=j_out_sb[:, k * IKS:(k + 1) * IKS],
                    in_=jt_out_d[k * HALF:(k + 1) * HALF, :],
                )

            ombd_sb = res.tile([HALF, 4 * HALF], FP32R, tag="ombd")
            for m in range(4):
                nc.sync.dma_start(
                    out=ombd_sb[:, m * HALF:(m + 1) * HALF],
                    in_=ombd_d[m * HALF:(m + 1) * HALF, :],
                )
            bones_r = res.tile([HALF, HALF], FP32R, tag="bones_r")
            bones_f = res.tile([HALF, HALF], FP32, tag="bones_f")
            nc.sync.dma_start(out=bones_r[:, :], in_=bones_d[:, :].bitcast(FP32R))
            nc.sync.dma_start(out=bones_f[:, :], in_=bones_d[:, :])

            embt_sb = res.tile([HALF, 2 * 2 * GCOL], FP32, tag="embt")
            for r in range(2):
                nc.sync.dma_start(
                    out=embt_sb[:, r * 2 * GCOL:(r + 1) * 2 * GCOL],
                    in_=embt_d[r * HALF:(r + 1) * HALF, :],
                )
            embt_own_sb = res.tile([HALF, 2 * GCOL], FP32, tag="embt_own")
            nc.sync.dma_start(out=embt_own_sb[:, :], in_=embt_own_d[:, :])
            bdt_sb = res.tile([HALF, 32], FP32, tag="bdt")
            nc.sync.dma_start(out=bdt_sb[:, :], in_=bdt_d[:, :])
            bdt_s_sb = res.tile([HALF, 4], FP32, tag="bdt_s")
            nc.sync.dma_start(out=bdt_s_sb[:, :], in_=bdt_s_d[:, :])

            # state slices (own ik rows), fp32 + rounded fp32r copy
            xs = res.tile([HALF, 8 * GCOL], FP32, tag="xs")
            xsr = res.tile([HALF, 8 * GCOL], FP32R, tag="xsr")
            cs = res.tile([HALF, 8 * GCOL], FP32, tag="cs")

            # ---------------- init: full X0 -> xg_t ----------------
            def normalize_into(u2, bones_tile, dt_rhs, out_sl_writer):
                """u2: [128,256] fp32 SBUF tile. Writes normalized result."""
                sq = tmp.tile([HALF, GCOL], dt_rhs, tag="sq")
                nc.vector.tensor_mul(out=sq[:, :], in0=u2[:, :], in1=u2[:, :])
                nb = auxps.tile([HALF, GCOL], FP32, tag="aux")
                nc.tensor.matmul(
                    nb[:, :], bones_tile[:, :], sq[:, :], start=True, stop=True
                )
                nrm = tmp.tile([HALF, GCOL], FP32, tag="nrm")
                nc.scalar.activation(nrm[:, :], nb[:, :], AF.Sqrt)
                rinv = tmp.tile([HALF, GCOL], FP32, tag="rinv")
                nc.vector.reciprocal(out=rinv[:, :], in_=nrm[:, :])
                out_sl_writer(u2, rinv)

            for g in range(2):
                for kk in range(32):
                    cps = auxps.tile([HALF, GCOL], FP32, tag="aux")
                    wt = stream.tile([HALF, HALF], FP32, tag="wdiag")
                    nc.sync.dma_start(
                        out=wt[:, :], in_=wdiag_d[kk * HALF:(kk + 1) * HALF, :]
                    )
                    nc.tensor.matmul(
                        cps[:, :],
                        wt[:, :],
                        embt_sb[:, (kk // 16) * 2 * GCOL + g * GCOL:][:, :GCOL],
                        start=True,
                        stop=True,
                    )
                    nt = stream.tile([HALF, GCOL], FP32, tag="noise")
                    nc.sync.dma_start(
                        out=nt[:, :],
                        in_=noiset_d[kk * HALF:(kk + 1) * HALF, g * GCOL:(g + 1) * GCOL],
                    )
                    u2 = tmp.tile([HALF, GCOL], FP32, tag="u2")
                    # u2 = (cps + b_d_col) + noise
                    nc.vector.scalar_tensor_tensor(
                        out=u2[:, :],
                        in0=cps[:, :],
                        scalar=bdt_sb[:, kk:kk + 1],
                        in1=nt[:, :],
                        op0=mybir.AluOpType.add,
                        op1=mybir.AluOpType.add,
                    )

                    def wr(u2_, rinv_, g=g, kk=kk):
                        x0 = tmp.tile([HALF, GCOL], FP16, tag="x0")
                        nc.vector.tensor_mul(out=x0[:, :], in0=u2_[:, :], in1=rinv_[:, :])
                        nc.sync.dma_start(
                            out=xg_t[g][kk * HALF:(kk + 1) * HALF, :], in_=x0[:, :]
                        )

                    normalize_into(u2, bones_f, FP32, wr)

            # ---------------- init: own-slice c_s and x_s ----------------
            for g in range(2):
                for m in range(4):
                    sl = slice((g * 4 + m) * GCOL, (g * 4 + m + 1) * GCOL)
                    cps = auxps.tile([HALF, GCOL], FP32, tag="aux")
                    wt = stream.tile([HALF, HALF], FP32, tag="wdiag")
                    nc.sync.dma_start(
                        out=wt[:, :], in_=wdiag_s_d[m * HALF:(m + 1) * HALF, :]
                    )
                    nc.tensor.matmul(
                        cps[:, :],
                        wt[:, :],
                        embt_own_sb[:, g * GCOL:(g + 1) * GCOL],
                        start=True,
                        stop=True,
                    )
                    nc.vector.tensor_scalar_add(
                        cs[:, sl], cps[:, :], bdt_s_sb[:, m:m + 1]
                    )
                    nt = stream.tile([HALF, GCOL], FP32, tag="noise")
                    nc.sync.dma_start(
                        out=nt[:, :],
                        in_=noiset_own_d[m * HALF:(m + 1) * HALF, g * GCOL:(g + 1) * GCOL],
                    )
                    u2 = tmp.tile([HALF, GCOL], FP32, tag="u2")
                    nc.vector.tensor_add(out=u2[:, :], in0=cs[:, sl], in1=nt[:, :])

                    def wr(u2_, rinv_, sl=sl):
                        nc.vector.tensor_mul(out=xs[:, sl], in0=u2_[:, :], in1=rinv_[:, :])
                        nc.vector.tensor_copy(out=xsr[:, sl], in_=xs[:, sl])

                    normalize_into(u2, bones_f, FP32, wr)

            # ---------------- main loop ----------------
            prev_cc = [None, None]
            for t in range(T):
                for g in range(2):
                    f01 = fps.tile([HALF, 2 * GCOL], FP32, tag=f"f{g}")
                    f23 = fps.tile([HALF, 2 * GCOL], FP32, tag=f"f{g}")

                    def freg(m):
                        ft = f01 if m < 2 else f23
                        c0 = (m % 2) * GCOL
                        return ft[:, c0:c0 + GCOL]

                    for k in range(32):
                        xk = xstream.tile([HALF, GCOL], FP16, tag=f"xg{g}")
                        dma = nc.sync.dma_start(
                            out=xk[:, :],
                            in_=xg_t[g][k * HALF:(k + 1) * HALF, :],
                        )
                        if prev_cc[g] is not None:
                            tile.add_dep_helper(
                                dma.ins, prev_cc[g].ins, reason="AG->stream RAW"
                            )
                        xk_sw = _swap_halves(xk[:, :])
                        for m in range(4):
                            # start=True clears the whole PSUM bank's
                            # has_written bits -- issue it only on the first
                            # matmul ever touching each bank (m=0/m=2, k=0).
                            nc.tensor.matmul(
                                freg(m),
                                j_in_sb[:, k * IKS + m * HALF:][:, :HALF],
                                xk[:, :],
                                start=(k == 0 and m % 2 == 0),
                                stop=False,
                                skip_group_check=True,
                            )
                            nc.tensor.matmul(
                                freg(m),
                                j_out_sb[:, k * IKS + m * HALF:][:, :HALF],
                                xk_sw,
                                start=False,
                                stop=False,
                                skip_group_check=True,
                            )

                    for m in range(4):
                        sl = slice((g * 4 + m) * GCOL, (g * 4 + m + 1) * GCOL)
                        # Omega rotation into same accumulation
                        nc.tensor.matmul(
                            freg(m),
                            ombd_sb[:, m * HALF:(m + 1) * HALF],
                            xsr[:, sl],
                            start=False,
                            stop=(m % 2 == 1),
                            skip_group_check=True,
                        )
                        h = tmp.tile([HALF, GCOL], FP32, tag="h")
                        nc.vector.tensor_add(out=h[:, :], in0=freg(m), in1=cs[:, sl])
                        if DBG and t == 0 and g == 0 and m == 0:
                            nc.sync.dma_start(out=dbg_h_d[:, :], in_=h[:, :])
                        xf = tmp.tile([HALF, GCOL], FP32R, tag="xf")
                        nc.vector.tensor_mul(out=xf[:, :], in0=h[:, :], in1=xs[:, sl])
                        sb_ps = auxps.tile([HALF, GCOL], FP32, tag="aux")
                        nc.tensor.matmul(
                            sb_ps[:, :], bones_r[:, :], xf[:, :], start=True, stop=True
                        )
                        t1 = tmp.tile([HALF, GCOL], FP32, tag="t1")
                        nc.vector.scalar_tensor_tensor(
                            out=t1[:, :],
                            in0=sb_ps[:, :],
                            scalar=GAMMA,
                            in1=xs[:, sl],
                            op0=mybir.AluOpType.mult,
                            op1=mybir.AluOpType.mult,
                        )
                        t2 = tmp.tile([HALF, GCOL], FP32, tag="t2")
                        nc.vector.scalar_tensor_tensor(
                            out=t2[:, :],
                            in0=h[:, :],
                            scalar=GAMMA,
                            in1=xs[:, sl],
                            op0=mybir.AluOpType.mult,
                            op1=mybir.AluOpType.add,
                        )
                        pre = tmp.tile([HALF, GCOL], FP32, tag="pre")
                        nc.vector.tensor_sub(out=pre[:, :], in0=t2[:, :], in1=t1[:, :])

                        def wr(pre_, rinv_, sl=sl, m=m, g=g, t=t):
                            nc.vector.tensor_mul(
                                out=xs[:, sl], in0=pre_[:, :], in1=rinv_[:, :]
                            )
                            nc.vector.tensor_copy(out=xsr[:, sl], in_=xs[:, sl])
                            if t < T - 1:
                                xh = tmp.tile([HALF, GCOL], FP16, tag="xh")
                                nc.vector.tensor_copy(out=xh[:, :], in_=xs[:, sl])
                                nc.sync.dma_start(
                                    out=agin[g][m * HALF:(m + 1) * HALF, :], in_=xh[:, :]
                                )

                        normalize_into(pre, bones_r, FP32R, wr)

                    if t < T - 1:
                        cc = nc.gpsimd.collective_compute(
                            "AllGather",
                            mybir.AluOpType.bypass,
                            replica_groups=[list(range(NCORES))],
                            ins=[agin[g][:, :].opt()],
                            outs=[xg_t[g][:, :].opt()],
                        )
                        prev_cc[g] = cc

            # ---------------- output: own ik rows, fp32 ----------------
            for g in range(2):
                for m in range(4):
                    sl = slice((g * 4 + m) * GCOL, (g * 4 + m + 1) * GCOL)
                    nc.sync.dma_start(
                        out=xt_out_d[m * HALF:(m + 1) * HALF, g * GCOL:(g + 1) * GCOL],
                        in_=xs[:, sl],
                    )

    nc.compile()
    return nc


def _get_nc():
    if "nc" not in _CACHE:
        nc = bacc.Bacc(
            "TRN2", target_bir_lowering=False, debug=False, num_devices=NCORES
        )
        _build(nc)
        nc.m = get_hw_module(nc.m)
        _CACHE["nc"] = nc
    return _CACHE["nc"]


def _marshal(embeddings1, embeddings2, W_d, b_d, J_in, J_out, Omega, noise1, noise2):
    """Host-side pure data movement: slice/transpose/scatter into device layout."""
    f32 = np.float32

    # J^T[(j,l),(i,k)] = J[i,j,k,l]
    jt_in = np.ascontiguousarray(J_in.transpose(1, 3, 0, 2).reshape(DN, DN))
    jt_out = np.ascontiguousarray(J_out.transpose(1, 3, 0, 2).reshape(DN, DN))

    # column layout: [x1_A | x2_A | x1_B | x2_B], 128 batches each
    def colcat(a1, a2):  # a1,a2: [B, ...] -> stacked columns
        return np.concatenate(
            [a1[:HALF], a2[:HALF], a1[HALF:], a2[HALF:]], axis=0
        )

    embt = np.ascontiguousarray(colcat(embeddings1, embeddings2).T)      # [256,512]
    nt1 = noise1.transpose(1, 2, 0).reshape(DN, B)                       # [(d,n), b]
    nt2 = noise2.transpose(1, 2, 0).reshape(DN, B)
    noiset = np.ascontiguousarray(
        np.concatenate([nt1[:, :HALF], nt2[:, :HALF], nt1[:, HALF:], nt2[:, HALF:]], axis=1)
    )

    bdt_flat = np.ascontiguousarray(b_d.reshape(DN))
    bdt = np.ascontiguousarray(bdt_flat.reshape(32, HALF).T)             # [128, 32]

    # W_diag scatter: block kk covers i in [8kk, 8kk+8)
    wdiag = np.zeros((32, HALF, HALF), f32)
    for kk in range(32):
        base_row = 128 * (kk // 16)
        for iloc in range(8):
            i = 8 * kk + iloc
            wdiag[kk, i - base_row, iloc * 16:(iloc + 1) * 16] = W_d[i]
    wdiag = wdiag.reshape(32 * HALF, HALF)

    bones = np.kron(np.eye(8, dtype=f32), np.ones((16, 16), f32))

    in_maps = []
    for c in range(NCORES):
        i0 = NIPC * c
        ik0 = IKS * c
        ombd = np.zeros((4, HALF, HALF), f32)
        for m in range(4):
            for iloc in range(8):
                i = i0 + 8 * m + iloc
                ombd[m, iloc * 16:(iloc + 1) * 16, iloc * 16:(iloc + 1) * 16] = Omega[i].T
        in_maps.append(
            {
                "jt_in": np.ascontiguousarray(jt_in[:, ik0:ik0 + IKS]).astype(np.float16),
                "jt_out": np.ascontiguousarray(jt_out[:, ik0:ik0 + IKS]).astype(np.float16),
                "ombd": ombd.reshape(4 * HALF, HALF),
                "wdiag": wdiag,
                "wdiag_s": np.ascontiguousarray(
                    wdiag.reshape(32, HALF, HALF)[4 * c:4 * c + 4].reshape(4 * HALF, HALF)
                ),
                "bones": bones,
                "embt": embt,
                "embt_own": np.ascontiguousarray(
                    embt[128 * (c // 4):128 * (c // 4) + 128]
                ),
                "bdt": bdt,
                "bdt_s": np.ascontiguousarray(
                    bdt_flat[ik0:ik0 + IKS].reshape(4, HALF).T
                ),
                "noiset": noiset,
                "noiset_own": np.ascontiguousarray(noiset[ik0:ik0 + IKS]),
            }
        )
    return in_maps


def _unmarshal(xt):
    """xt: [4096, 512] -> [2, B, D, N]"""
    x1 = np.concatenate([xt[:, 0:HALF], xt[:, 2 * HALF:3 * HALF]], axis=1)
    x2 = np.concatenate([xt[:, HALF:2 * HALF], xt[:, 3 * HALF:4 * HALF]], axis=1)
    out = np.empty((2, B, D, N), np.float32)
    out[0] = x1.reshape(D, N, B).transpose(2, 0, 1)
    out[1] = x2.reshape(D, N, B).transpose(2, 0, 1)
    return out


def run_on_device(in_maps):
    nc = _get_nc()
    return bass2jax.run_bass_via_pjrt(nc, in_maps, n_cores=NCORES)


def kernel(**inputs):
    in_maps = _marshal(**{k: np.asarray(v, np.float32) for k, v in inputs.items()})
    results = run_on_device(in_maps)
    xt = np.concatenate([results[c]["xt_out"] for c in range(NCORES)], axis=0)
    return _unmarshal(xt)


if __name__ == "__main__":
    rng = np.random.default_rng(0)
    ins = {
        "embeddings1": rng.standard_normal((B, D), dtype=np.float32),
        "embeddings2": rng.standard_normal((B, D), dtype=np.float32),
        "W_d": rng.standard_normal((D, N), dtype=np.float32) * 0.1,
        "b_d": np.zeros((D, N), np.float32),
        "J_in": (rng.standard_normal((D, D, N, N), dtype=np.float32) * 0.007),
        "J_out": (rng.standard_normal((D, D, N, N), dtype=np.float32) * 0.007),
        "Omega": rng.standard_normal((D, N, N), dtype=np.float32) * 0.1,
        "noise1": rng.standard_normal((B, D, N), dtype=np.float32) * 0.05,
        "noise2": rng.standard_normal((B, D, N), dtype=np.float32) * 0.05,
    }
    t0 = time.time()
    out = kernel(**ins)
    print("kernel() took", time.time() - t0, "s; out shape", out.shape)


# revision 8
# speedup vs baseline: 1.2307x; 1.0463x over previous
"""Kuramoto oscillator network kernel for 8 Trainium2 NeuronCores.

Problem: B=256 batches, D=256 feature dims, N=16 oscillator dims, T=25 steps.
    c = emb[:,:,None]*W_d + b_d                        [B,D,N]
    x = normalize(noise + c)                            (init, per (b,d) over N)
    repeat T: f1 = J_in@x1 + J_out@x2 + c1  (einsum ijkl,bjl->bik)
              p  = f - <x,f>x ; om = Omega@x
              x  = normalize(x + g*(om + p))
    out = stack(x1, x2)                                 [2,B,D,N]

Strategy (model-parallel over output dim i, all-transposed layout):
  * Each core owns a 512-wide slice of the flattened ik axis (32 of 256 i's).
    J_in^T / J_out^T slices ([4096 x 512] each) stay resident in SBUF and are
    used as matmul stationary tiles in float32r (FP22 read truncation, full
    bf16-rate on the PE vs 4x slower true fp32).
  * State X^T [(j,l), batchcol] lives in HBM, AllGather'd across cores each
    step; columns are [x1|x2] so the J_out cross-coupling is a column-swapped
    rhs view (negative-stride AP).
  * Omega rotation is 4 extra block-diagonal matmuls accumulated into the same
    PSUM as f (skew-symmetry makes <x,Omega x>=0, so the tangent projection is
    unaffected).
  * Per-(b,i) reductions over the 16 oscillator partitions (projection <x,f>
    and the normalize norm) are single matmuls against a block-ones matrix
    that reduce AND broadcast in one shot.
  * Batches split into two groups (A=0:128, B=128:256) pipelined so each
    group's AllGather hides under the other group's matmul phase.

Self-contained: hardcodes shapes; no imports from /root/problem.
"""

import os
import sys
import time

sys.path.insert(0, "/opt/trn_rl_repo")

import numpy as np

import concourse.bass as bass
import concourse.mybir as mybir
import concourse.tile as tile
from concourse import bacc
from concourse import bass2jax
from concourse.bass_interp import get_hw_module

B, D, N = 256, 256, 16
DN = D * N                      # 4096 flattened (i,k) / (j,l)
T = int(os.environ.get("KUR_T", "25"))
GAMMA = 0.1
NCORES = 8
IKS = DN // NCORES              # 512 ik per core (32 i values)
NIPC = D // NCORES              # 32 i per core
GCOL = 256                      # columns per batch group (128 x1 + 128 x2)
HALF = 128

FP32 = mybir.dt.float32
FP32R = mybir.dt.float32r
FP16 = mybir.dt.float16

_CACHE = {}


def _swap_halves(ap):
    """View a [128, 256] SBUF AP with its two 128-column halves swapped."""
    return bass.AP(
        tensor=ap.tensor,
        offset=ap.offset + HALF,
        ap=[list(ap.ap[0])] + [[-HALF, 2], [1, HALF]],
    )


def _build(nc):
    AF = mybir.ActivationFunctionType

    # ---------------- DRAM I/O ----------------
    jt_in_d = nc.dram_tensor("jt_in", [DN, IKS], FP16, kind="ExternalInput")
    jt_out_d = nc.dram_tensor("jt_out", [DN, IKS], FP16, kind="ExternalInput")
    ombd_d = nc.dram_tensor("ombd", [4 * HALF, HALF], FP32R, kind="ExternalInput")
    wdiag_d = nc.dram_tensor("wdiag", [32 * HALF, HALF], FP32, kind="ExternalInput")
    wdiag_s_d = nc.dram_tensor("wdiag_s", [4 * HALF, HALF], FP32, kind="ExternalInput")
    bones_d = nc.dram_tensor("bones", [HALF, HALF], FP32, kind="ExternalInput")
    embt_d = nc.dram_tensor("embt", [2 * HALF, 2 * GCOL], FP32, kind="ExternalInput")
    embt_own_d = nc.dram_tensor("embt_own", [HALF, 2 * GCOL], FP32, kind="ExternalInput")
    bdt_d = nc.dram_tensor("bdt", [HALF, 32], FP32, kind="ExternalInput")
    bdt_s_d = nc.dram_tensor("bdt_s", [HALF, 4], FP32, kind="ExternalInput")
    noiset_d = nc.dram_tensor("noiset", [DN, 2 * GCOL], FP32, kind="ExternalInput")
    noiset_own_d = nc.dram_tensor("noiset_own", [IKS, 2 * GCOL], FP32, kind="ExternalInput")

    xt_out_d = nc.dram_tensor("xt_out", [IKS, 2 * GCOL], FP32, kind="ExternalOutput")
    DBG = os.environ.get("KUR_DBG", "0") == "1"
    if DBG:
        dbg_h_d = nc.dram_tensor("dbg_h", [HALF, GCOL], FP32, kind="ExternalOutput")

    # internal HBM: gathered state + AG input bounce, per group
    xg_t = [
        nc.dram_tensor(f"xg{g}_t", [DN, GCOL], FP16, addr_space="Shared")
        for g in range(2)
    ]
    agin = [nc.dram_tensor(f"agin{g}", [IKS, GCOL], FP16) for g in range(2)]

    with tile.TileContext(nc) as tc:
        with (
            tc.tile_pool(name="res", bufs=1) as res,
            tc.tile_pool(name="stream", bufs=4) as stream,
            tc.tile_pool(name="xstream", bufs=8) as xstream,
            tc.tile_pool(name="tmp", bufs=2) as tmp,
            tc.tile_pool(name="fps", bufs=2, space="PSUM") as fps,
            tc.tile_pool(name="auxps", bufs=3, space="PSUM") as auxps,
            tc.tile_pool(name="dram", bufs=1, space="DRAM") as _dr,
        ):
            # ---------------- resident SBUF ----------------
            j_in_sb = res.tile([HALF, 32 * IKS], FP16, tag="jin")
            j_out_sb = res.tile([HALF, 32 * IKS], FP16, tag="jout")
            for k in range(32):
                nc.sync.dma_start(
                    out=j_in_sb[:, k * IKS:(k + 1) * IKS],
                    in_=jt_in_d[k * HALF:(k + 1) * HALF, :],
                )
                nc.sync.dma_start(
                    out=j_out_sb[:, k * IKS:(k + 1) * IKS],
                    in_=jt_out_d[k * HALF:(k + 1) * HALF, :],
                )

            ombd_sb = res.tile([HALF, 4 * HALF], FP32R, tag="ombd")
            for m in range(4):
                nc.sync.dma_start(
                    out=ombd_sb[:, m * HALF:(m + 1) * HALF],
                    in_=ombd_d[m * HALF:(m + 1) * HALF, :],
                )
            bones_r = res.tile([HALF, HALF], FP32R, tag="bones_r")
            bones_f = res.tile([HALF, HALF], FP32, tag="bones_f")
            nc.sync.dma_start(out=bones_r[:, :], in_=bones_d[:, :].bitcast(FP32R))
            nc.sync.dma_start(out=bones_f[:, :], in_=bones_d[:, :])

            embt_sb = res.tile([HALF, 2 * 2 * GCOL], FP32, tag="embt")
            for r in range(2):
                nc.sync.dma_start(
                    out=embt_sb[:, r * 2 * GCOL:(r + 1) * 2 * GCOL],
                    in_=embt_d[r * HALF:(r + 1) * HALF, :],
                )
            embt_own_sb = res.tile([HALF, 2 * GCOL], FP32, tag="embt_own")
            nc.sync.dma_start(out=embt_own_sb[:, :], in_=embt_own_d[:, :])
            bdt_sb = res.tile([HALF, 32], FP32, tag="bdt")
            nc.sync.dma_start(out=bdt_sb[:, :], in_=bdt_d[:, :])
            bdt_s_sb = res.tile([HALF, 4], FP32, tag="bdt_s")
            nc.sync.dma_start(out=bdt_s_sb[:, :], in_=bdt_s_d[:, :])

            # state slices (own ik rows), fp32 + rounded fp32r copy
            xs = res.tile([HALF, 8 * GCOL], FP32, tag="xs")
            xsr = res.tile([HALF, 8 * GCOL], FP32R, tag="xsr")
            cs = res.tile([HALF, 8 * GCOL], FP32, tag="cs")

            # ---------------- init: full X0 -> xg_t ----------------
            def normalize_into(u2, bones_tile, dt_rhs, out_sl_writer):
                """u2: [128,256] fp32 SBUF tile. Writes normalized result."""
                sq = tmp.tile([HALF, GCOL], dt_rhs, tag="sq")
                nc.vector.tensor_mul(out=sq[:, :], in0=u2[:, :], in1=u2[:, :])
                nb = auxps.tile([HALF, GCOL], FP32, tag="aux")
                nc.tensor.matmul(
                    nb[:, :], bones_tile[:, :], sq[:, :], start=True, stop=True
                )
                nrm = tmp.tile([HALF, GCOL], FP32, tag="nrm")
                nc.scalar.activation(nrm[:, :], nb[:, :], AF.Sqrt)
                rinv = tmp.tile([HALF, GCOL], FP32, tag="rinv")
                nc.vector.reciprocal(out=rinv[:, :], in_=nrm[:, :])
                out_sl_writer(u2, rinv)

            for g in range(2):
                for kk in range(32):
                    cps = auxps.tile([HALF, GCOL], FP32, tag="aux")
                    wt = stream.tile([HALF, HALF], FP32, tag="wdiag")
                    nc.sync.dma_start(
                        out=wt[:, :], in_=wdiag_d[kk * HALF:(kk + 1) * HALF, :]
                    )
                    nc.tensor.matmul(
                        cps[:, :],
                        wt[:, :],
                        embt_sb[:, (kk // 16) * 2 * GCOL + g * GCOL:][:, :GCOL],
                        start=True,
                        stop=True,
                    )
                    nt = stream.tile([HALF, GCOL], FP32, tag="noise")
                    nc.sync.dma_start(
                        out=nt[:, :],
                        in_=noiset_d[kk * HALF:(kk + 1) * HALF, g * GCOL:(g + 1) * GCOL],
                    )
                    u2 = tmp.tile([HALF, GCOL], FP32, tag="u2")
                    # u2 = (cps + b_d_col) + noise
                    nc.vector.scalar_tensor_tensor(
                        out=u2[:, :],
                        in0=cps[:, :],
                        scalar=bdt_sb[:, kk:kk + 1],
                        in1=nt[:, :],
                        op0=mybir.AluOpType.add,
                        op1=mybir.AluOpType.add,
                    )

                    def wr(u2_, rinv_, g=g, kk=kk):
                        x0 = tmp.tile([HALF, GCOL], FP16, tag="x0")
                        nc.vector.tensor_mul(out=x0[:, :], in0=u2_[:, :], in1=rinv_[:, :])
                        nc.sync.dma_start(
                            out=xg_t[g][kk * HALF:(kk + 1) * HALF, :], in_=x0[:, :]
                        )

                    normalize_into(u2, bones_f, FP32, wr)

            # ---------------- init: own-slice c_s and x_s ----------------
            for g in range(2):
                for m in range(4):
                    sl = slice((g * 4 + m) * GCOL, (g * 4 + m + 1) * GCOL)
                    cps = auxps.tile([HALF, GCOL], FP32, tag="aux")
                    wt = stream.tile([HALF, HALF], FP32, tag="wdiag")
                    nc.sync.dma_start(
                        out=wt[:, :], in_=wdiag_s_d[m * HALF:(m + 1) * HALF, :]
                    )
                    nc.tensor.matmul(
                        cps[:, :],
                        wt[:, :],
                        embt_own_sb[:, g * GCOL:(g + 1) * GCOL],
                        start=True,
                        stop=True,
                    )
                    nc.vector.tensor_scalar_add(
                        cs[:, sl], cps[:, :], bdt_s_sb[:, m:m + 1]
                    )
                    nt = stream.tile([HALF, GCOL], FP32, tag="noise")
                    nc.sync.dma_start(
                        out=nt[:, :],
                        in_=noiset_own_d[m * HALF:(m + 1) * HALF, g * GCOL:(g + 1) * GCOL],
                    )
                    u2 = tmp.tile([HALF, GCOL], FP32, tag="u2")
                    nc.vector.tensor_add(out=u2[:, :], in0=cs[:, sl], in1=nt[:, :])

                    def wr(u2_, rinv_, sl=sl):
                        nc.vector.tensor_mul(out=xs[:, sl], in0=u2_[:, :], in1=rinv_[:, :])
                        nc.vector.tensor_copy(out=xsr[:, sl], in_=xs[:, sl])

                    normalize_into(u2, bones_f, FP32, wr)

            # ---------------- main loop ----------------
            prev_cc = [None, None]
            for t in range(T):
                for g in range(2):
                    f01 = fps.tile([HALF, 2 * GCOL], FP32, tag=f"f{g}")
                    f23 = fps.tile([HALF, 2 * GCOL], FP32, tag=f"f{g}")

                    def freg(m):
                        ft = f01 if m < 2 else f23
                        c0 = (m % 2) * GCOL
                        return ft[:, c0:c0 + GCOL]

                    for k in range(32):
                        xk = xstream.tile([HALF, GCOL], FP16, tag=f"xg{g}")
                        dma = nc.sync.dma_start(
                            out=xk[:, :],
                            in_=xg_t[g][k * HALF:(k + 1) * HALF, :],
                        )
                        if prev_cc[g] is not None:
                            tile.add_dep_helper(
                                dma.ins, prev_cc[g].ins, reason="AG->stream RAW"
                            )
                        xk_sw = _swap_halves(xk[:, :])
                        for m in range(4):
                            # start=True clears the whole PSUM bank's
                            # has_written bits -- issue it only on the first
                            # matmul ever touching each bank (m=0/m=2, k=0).
                            nc.tensor.matmul(
                                freg(m),
                                j_in_sb[:, k * IKS + m * HALF:][:, :HALF],
                                xk[:, :],
                                start=(k == 0 and m % 2 == 0),
                                stop=False,
                                skip_group_check=True,
                            )
                            nc.tensor.matmul(
                                freg(m),
                                j_out_sb[:, k * IKS + m * HALF:][:, :HALF],
                                xk_sw,
                                start=False,
                                stop=False,
                                skip_group_check=True,
                            )

                    for m in range(4):
                        sl = slice((g * 4 + m) * GCOL, (g * 4 + m + 1) * GCOL)
                        # Omega rotation into same accumulation
                        nc.tensor.matmul(
                            freg(m),
                            ombd_sb[:, m * HALF:(m + 1) * HALF],
                            xsr[:, sl],
                            start=False,
                            stop=(m % 2 == 1),
                            skip_group_check=True,
                        )
                        h = tmp.tile([HALF, GCOL], FP32, tag="h")
                        nc.vector.tensor_add(out=h[:, :], in0=freg(m), in1=cs[:, sl])
                        if DBG and t == 0 and g == 0 and m == 0:
                            nc.sync.dma_start(out=dbg_h_d[:, :], in_=h[:, :])
                        xf = tmp.tile([HALF, GCOL], FP32R, tag="xf")
                        nc.vector.tensor_mul(out=xf[:, :], in0=h[:, :], in1=xs[:, sl])
                        sb_ps = auxps.tile([HALF, GCOL], FP32, tag="aux")
                        nc.tensor.matmul(
                            sb_ps[:, :], bones_r[:, :], xf[:, :], start=True, stop=True
                        )
                        t1 = tmp.tile([HALF, GCOL], FP32, tag="t1")
                        nc.vector.scalar_tensor_tensor(
                            out=t1[:, :],
                            in0=sb_ps[:, :],
                            scalar=GAMMA,
                            in1=xs[:, sl],
                            op0=mybir.AluOpType.mult,
                            op1=mybir.AluOpType.mult,
                        )
                        t2 = tmp.tile([HALF, GCOL], FP32, tag="t2")
                        nc.vector.scalar_tensor_tensor(
                            out=t2[:, :],
                            in0=h[:, :],
                            scalar=GAMMA,
                            in1=xs[:, sl],
                            op0=mybir.AluOpType.mult,
                            op1=mybir.AluOpType.add,
                        )
                        pre = tmp.tile([HALF, GCOL], FP32, tag="pre")
                        nc.vector.tensor_sub(out=pre[:, :], in0=t2[:, :], in1=t1[:, :])

                        def wr(pre_, rinv_, sl=sl, m=m, g=g, t=t):
                            nc.vector.tensor_mul(
                                out=xs[:, sl], in0=pre_[:, :], in1=rinv_[:, :]
                            )
                            nc.vector.tensor_copy(out=xsr[:, sl], in_=xs[:, sl])
                            if t < T - 1:
                                xh = tmp.tile([HALF, GCOL], FP16, tag="xh")
                                nc.vector.tensor_copy(out=xh[:, :], in_=xs[:, sl])
                                nc.sync.dma_start(
                                    out=agin[g][m * HALF:(m + 1) * HALF, :], in_=xh[:, :]
                                )

                        normalize_into(pre, bones_r, FP32R, wr)

                    if t < T - 1:
                        cc = nc.gpsimd.collective_compute(
                            "AllGather",
                            mybir.AluOpType.bypass,
                            replica_groups=[list(range(NCORES))],
                            ins=[agin[g][:, :].opt()],
                            outs=[xg_t[g][:, :].opt()],
                        )
                        prev_cc[g] = cc

            # ---------------- output: own ik rows, fp32 ----------------
            for g in range(2):
                for m in range(4):
                    sl = slice((g * 4 + m) * GCOL, (g * 4 + m + 1) * GCOL)
                    nc.sync.dma_start(
                        out=xt_out_d[m * HALF:(m + 1) * HALF, g * GCOL:(g + 1) * GCOL],
                        in_=xs[:, sl],
                    )

    nc.compile()
    return nc


def _get_nc():
    if "nc" not in _CACHE:
        nc = bacc.Bacc(
            "TRN2", target_bir_lowering=False, debug=False, num_devices=NCORES
        )
        _build(nc)
        nc.m = get_hw_module(nc.m)
        _CACHE["nc"] = nc
    return _CACHE["nc"]


def _marshal(embeddings1, embeddings2, W_d, b_d, J_in, J_out, Omega, noise1, noise2):
    """Host-side pure data movement: slice/transpose/scatter into device layout."""
    f32 = np.float32

    # J^T[(j,l),(i,k)] = J[i,j,k,l]
    jt_in = np.ascontiguousarray(J_in.transpose(1, 3, 0, 2).reshape(DN, DN))
    jt_out = np.ascontiguousarray(J_out.transpose(1, 3, 0, 2).reshape(DN, DN))

    # column layout: [x1_A | x2_A | x1_B | x2_B], 128 batches each
    def colcat(a1, a2):  # a1,a2: [B, ...] -> stacked columns
        return np.concatenate(
            [a1[:HALF], a2[:HALF], a1[HALF:], a2[HALF:]], axis=0
        )

    embt = np.ascontiguousarray(colcat(embeddings1, embeddings2).T)      # [256,512]
    nt1 = noise1.transpose(1, 2, 0).reshape(DN, B)                       # [(d,n), b]
    nt2 = noise2.transpose(1, 2, 0).reshape(DN, B)
    noiset = np.ascontiguousarray(
        np.concatenate([nt1[:, :HALF], nt2[:, :HALF], nt1[:, HALF:], nt2[:, HALF:]], axis=1)
    )

    bdt_flat = np.ascontiguousarray(b_d.reshape(DN))
    bdt = np.ascontiguousarray(bdt_flat.reshape(32, HALF).T)             # [128, 32]

    # W_diag scatter: block kk covers i in [8kk, 8kk+8)
    wdiag = np.zeros((32, HALF, HALF), f32)
    for kk in range(32):
        base_row = 128 * (kk // 16)
        for iloc in range(8):
            i = 8 * kk + iloc
            wdiag[kk, i - base_row, iloc * 16:(iloc + 1) * 16] = W_d[i]
    wdiag = wdiag.reshape(32 * HALF, HALF)

    bones = np.kron(np.eye(8, dtype=f32), np.ones((16, 16), f32))

    in_maps = []
    for c in range(NCORES):
        i0 = NIPC * c
        ik0 = IKS * c
        ombd = np.zeros((4, HALF, HALF), f32)
        for m in range(4):
            for iloc in range(8):
                i = i0 + 8 * m + iloc
                ombd[m, iloc * 16:(iloc + 1) * 16, iloc * 16:(iloc + 1) * 16] = Omega[i].T
        in_maps.append(
            {
                "jt_in": np.ascontiguousarray(jt_in[:, ik0:ik0 + IKS]).astype(np.float16),
                "jt_out": np.ascontiguousarray(jt_out[:, ik0:ik0 + IKS]).astype(np.float16),
                "ombd": ombd.reshape(4 * HALF, HALF),
                "wdiag": wdiag,
                "wdiag_s": np.ascontiguousarray(
                    wdiag.reshape(32, HALF, HALF)[4 * c:4 * c + 4].reshape(4 * HALF, HALF)
                ),
                "bones": bones,
                "embt": embt,
                "embt_own": np.ascontiguousarray(
                    embt[128 * (c // 4):128 * (c // 4) + 128]
                ),
                "bdt": bdt,
                "bdt_s": np.ascontiguousarray(
                    bdt_flat[ik0:ik0 + IKS].reshape(4, HALF).T
                ),
                "noiset": noiset,
                "noiset_own": np.ascontiguousarray(noiset[ik0:ik0 + IKS]),
            }
        )
    return in_maps


def _unmarshal(xt):
    """xt: [4096, 512] -> [2, B, D, N]"""
    x1 = np.concatenate([xt[:, 0:HALF], xt[:, 2 * HALF:3 * HALF]], axis=1)
    x2 = np.concatenate([xt[:, HALF:2 * HALF], xt[:, 3 * HALF:4 * HALF]], axis=1)
    out = np.empty((2, B, D, N), np.float32)
    out[0] = x1.reshape(D, N, B).transpose(2, 0, 1)
    out[1] = x2.reshape(D, N, B).transpose(2, 0, 1)
    return out


def run_on_device(in_maps):
    nc = _get_nc()
    return bass2jax.run_bass_via_pjrt(nc, in_maps, n_cores=NCORES)


def kernel(**inputs):
    in_maps = _marshal(**{k: np.asarray(v, np.float32) for k, v in inputs.items()})
    results = run_on_device(in_maps)
    xt = np.concatenate([results[c]["xt_out"] for c in range(NCORES)], axis=0)
    return _unmarshal(xt)


if __name__ == "__main__":
    rng = np.random.default_rng(0)
    ins = {
        "embeddings1": rng.standard_normal((B, D), dtype=np.float32),
        "embeddings2": rng.standard_normal((B, D), dtype=np.float32),
        "W_d": rng.standard_normal((D, N), dtype=np.float32) * 0.1,
        "b_d": np.zeros((D, N), np.float32),
        "J_in": (rng.standard_normal((D, D, N, N), dtype=np.float32) * 0.007),
        "J_out": (rng.standard_normal((D, D, N, N), dtype=np.float32) * 0.007),
        "Omega": rng.standard_normal((D, N, N), dtype=np.float32) * 0.1,
        "noise1": rng.standard_normal((B, D, N), dtype=np.float32) * 0.05,
        "noise2": rng.standard_normal((B, D, N), dtype=np.float32) * 0.05,
    }
    t0 = time.time()
    out = kernel(**ins)
    print("kernel() took", time.time() - t0, "s; out shape", out.shape)
